# revision 8
# baseline (speedup 1.0000x reference)
"""Trainium2 Bass kernel for single-head attention.

Problem: query [8192, 256], key [8192, 256], value [8192, 256] (fp32)
  out = softmax(Q @ K.T / sqrt(256)) @ V        -> [8192, 256]

Sharding: query rows split across 8 NeuronCores (1024 rows each);
K / V replicated. Each core computes its row-block independently.

Per-core algorithm (core c):
  - Layout trick: compute S^T [k, q] instead of S [q, k] so that the
    PV matmul needs no transpose:  S^T tile = (K chunk) @ (Q chunk)^T via
    PE matmul with d (head dim) on the contraction/partition axis:
        lhsT = K^T[d_chunk, k_block] (128x128), rhs = Q^T[d_chunk, q_block]
  - Scores ~ N(0,1) after the 1/16 scale, so exp() without max-subtraction
    is numerically safe (max score over 8192 samples ~ 4; exp(4) = 55).
  - P^T = exp(S^T / 16) on the ACT engine (scale fused), bf16 output.
    Two k-blocks' score tiles live in one PSUM pair tile [128, 2, 512]
    (adjacent banks) so ONE ACTIVATE covers both: halves ACT instruction
    count and gives the st-tile WAR chain ~1.2us of slack per super-block
    (separate 688ns exps had ~100ns slack -> periodic 432ns PE stalls).
  - O accumulation: out[q, v] = sum_k P^T[k, q]^T @ Vext[k, v] where Vext
    has a ones column appended -> column 256 accumulates the softmax
    denominator sum_k p. One PSUM accumulation group over all 64 k-blocks.
  - Normalize: O[:, 0:256] * (1 / O[:, 256]) per partition row, DMA out.

All matmul operands are bf16: same PE stream rate as float32r (1 col/
cycle) but LDWEIGHTS gets fast-weight-load (fp32r's fp32_mode=HIGH
disables FWL: ~12-18ns/matmul extra) and input DMA bytes halve.
Measured rms relative error ~3.3e-3 (tolerance 2e-2).

A dummy-matmul warmup chain runs during the initial DMA wait (~7.7-12us)
so the PE HAM clock-gate (cold 1.2 GHz -> warm 2.4 GHz after ~1-2 busy
windows of 3.4us) un-throttles before real work, and the PE is never
idle ahead of it.
"""
import numpy as np
import ml_dtypes
from contextlib import ExitStack

import concourse.bacc as bacc
import concourse.mybir as mybir
import concourse.tile as tile
from concourse import bass_utils

N, M, D, DV = 8192, 8192, 256, 256
NCORES = 8
QSH = N // NCORES        # 1024 query rows per core
QB = 512                 # q block (matmul moving free dim; one PSUM bank)
NQB = QSH // QB          # 2
KB = 128                 # k block (PE partition dim)
NKB = M // KB            # 64
SCALE = 1.0 / 16.0       # 1/sqrt(D)
DCH = D // 128           # 2 chunks of the contraction (head) dim
VW = DV + 2              # V columns + ones (denominator) + pad

_NC = None


def _build():
    f32 = mybir.dt.float32
    bf16 = mybir.dt.bfloat16

    nc = bacc.Bacc("TRN2", target_bir_lowering=False, debug=False)
    qT = nc.dram_tensor("qT", [D, QSH], bf16, kind="ExternalInput")
    kT = nc.dram_tensor("kT", [D, M], bf16, kind="ExternalInput")
    # V pre-permuted host-side to partition-major [128, NKB, 258] so DMA
    # lines are (b-range x 258) contiguous.
    vP = nc.dram_tensor("vP", [128, NKB, VW], bf16, kind="ExternalInput")
    o = nc.dram_tensor("o", [QSH, DV], f32, kind="ExternalOutput")

    kT_r = kT.ap().rearrange("(c p) k -> p c k", p=128)    # [128, 2, 8192]
    qT_r = qT.ap().rearrange("(c p) q -> p c q", p=128)    # [128, 2, 1024]
    v_r = vP.ap()                                          # [128, 64, 258]

    with tile.TileContext(nc) as tc, ExitStack() as ctx:
        sb = ctx.enter_context(tc.tile_pool(name="sb", bufs=1))
        pp = ctx.enter_context(tc.tile_pool(name="pp", bufs=4))
        outp = ctx.enter_context(tc.tile_pool(name="outp", bufs=2))
        # st pair tiles: [128, 2, 512] f32 = 2 adjacent PSUM banks; slice
        # [:, j, :] is exactly one bank (a legal matmul dst). 2 bufs = 4 banks.
        ps_st = ctx.enter_context(tc.tile_pool(name="ps_st", bufs=2, space="PSUM"))
        ps_o = ctx.enter_context(tc.tile_pool(name="ps_o", bufs=1, space="PSUM"))

        kt_sb = sb.tile([128, DCH, M], bf16, tag="kt")
        qt_sb = sb.tile([128, DCH, QSH], bf16, tag="qt")
        v_sb = sb.tile([128, NKB, VW], bf16, tag="v")
        warm = sb.tile([128, 256], bf16, tag="warm")

        # PE warmup: accumulation chain with no DMA dependencies, emitted
        # first so it runs during the initial DMA wait (~7.7us preamble end
        # -> ~12us first data) and starts the HAM activity window early.
        # Its PSUM comes from the ps_st rotation; SB0's second pair tile
        # carries a WAR dep on it -- satisfied long before that QK runs.
        nc.vector.memset(warm, 0.0)
        warm_ps = ps_st.tile([128, 2, QB], f32, tag="st", name="warm_ps")
        NWARM = 18
        for i in range(NWARM):
            nc.tensor.matmul(
                warm_ps[:, 0, 0:256], lhsT=warm[:, 0:128], rhs=warm,
                start=(i == 0), stop=(i == NWARM - 1),
            )

        # Input streaming, in consumption order, issued from two idle engine
        # queues in parallel (each DMA_DIRECT2D occupies its queue
        # ~0.7-1.5us): Sync feeds kt, GpSimd feeds qt + v. DGE packets only
        # start flowing ~2.5us after the first issue, then ~300 GB/s.
        nc.sync.dma_start(out=kt_sb[:, :, 0:512], in_=kT_r[:, :, 0:512])
        nc.gpsimd.dma_start(out=qt_sb[:, :, 0:QB], in_=qT_r[:, :, 0:QB])
        nc.sync.dma_start(out=kt_sb[:, :, 512:1024], in_=kT_r[:, :, 512:1024])
        nc.gpsimd.dma_start(out=v_sb[:, 0:4, :], in_=v_r[:, 0:4, :])
        nc.sync.dma_start(out=kt_sb[:, :, 1024:2048], in_=kT_r[:, :, 1024:2048])
        nc.gpsimd.dma_start(out=v_sb[:, 4:12, :], in_=v_r[:, 4:12, :])
        nc.sync.dma_start(out=kt_sb[:, :, 2048:4096], in_=kT_r[:, :, 2048:4096])
        nc.gpsimd.dma_start(out=v_sb[:, 12:28, :], in_=v_r[:, 12:28, :])
        nc.sync.dma_start(out=kt_sb[:, :, 4096:8192], in_=kT_r[:, :, 4096:8192])
        nc.gpsimd.dma_start(out=qt_sb[:, :, QB:QSH], in_=qT_r[:, :, QB:QSH])
        nc.gpsimd.dma_start(out=v_sb[:, 28:44, :], in_=v_r[:, 28:44, :])
        nc.gpsimd.dma_start(out=v_sb[:, 44:64, :], in_=v_r[:, 44:64, :])

        SB = 4  # kb super-block: longer same-type PE runs, fewer transitions
        NS = QB // 128
        NPAIR = SB // 2
        for qb in range(NQB):
            qsl = slice(qb * QB, (qb + 1) * QB)
            o_ps = [ps_o.tile([128, VW], f32, tag=f"o{s}", name=f"o_ps{s}") for s in range(NS)]

            def emit_pv_half(pts, kb0, pr, last):
                # s-major in the last half of the last super-block: each o_ps
                # accumulation closes as early as possible, so normalization +
                # output DMA overlap the remaining PV matmuls.
                js = (pr * 2, pr * 2 + 1)
                if last:
                    order = [(j, s) for s in range(NS) for j in js]
                else:
                    order = [(j, s) for j in js for s in range(NS)]
                for j, s in order:
                    kb = kb0 + j
                    nc.tensor.matmul(
                        o_ps[s],
                        lhsT=pts[j // 2][:, j % 2, s * 128:(s + 1) * 128],
                        rhs=v_sb[:, kb, :],
                        start=(kb == 0),
                        stop=(kb == NKB - 1),
                    )

            def emit_qk_pair(st, kb0, pr):
                # Interleave the pair's two k-blocks: consecutive PE matmuls
                # hit different PSUM banks (avoids same-bank accumulate
                # turnaround between a group's start and stop matmul).
                for c in range(DCH):
                    for j in (0, 1):
                        kb = kb0 + pr * 2 + j
                        ksl = slice(kb * KB, (kb + 1) * KB)
                        nc.tensor.matmul(
                            st[:, j, :],
                            lhsT=kt_sb[:, c, ksl],
                            rhs=qt_sb[:, c, qsl],
                            start=(c == 0),
                            stop=(c == DCH - 1),
                        )

            # Software-pipelined emission, interleaved at PAIR granularity:
            #   QKa(i) exp_a(i) PVa(i-1) QKb(i) exp_b(i) PVb(i-1)
            # Two purposes: (1) by the time PE reaches a PV group its exp
            # finished ~a super-block ago; (2) each wait-carrying QK start
            # matmul sits ~900ns after its exp semaphore arrives -- the PE NX
            # decodes ~860ns ahead, and a wait unsatisfied at decode time
            # breaks the matmul pipeline for ~2 issue slots (measured 432ns).
            prev_pv = None
            for kb0 in range(0, NKB, SB):
                sts = [
                    ps_st.tile([128, 2, QB], f32, tag="st", name="st")
                    for _ in range(NPAIR)
                ]
                for pr in range(NPAIR):
                    emit_qk_pair(sts[pr], kb0, pr)
                    p_t = pp.tile([128, 2, QB], bf16, tag="p", name="p_t")
                    nc.scalar.activation(
                        out=p_t, in_=sts[pr],
                        func=mybir.ActivationFunctionType.Exp, scale=SCALE,
                    )
                    if prev_pv is not None:
                        emit_pv_half(prev_pv[0], prev_pv[1], pr, last=False)
                    if pr == 0:
                        pts = []
                    pts.append(p_t)
                prev_pv = (pts, kb0)
            for pr in range(NPAIR):
                emit_pv_half(prev_pv[0], prev_pv[1], pr, last=True)
            # Normalize: recip on DVE as each o_ps closes. For qb0 keep ALL
            # multiplies on DVE (ACT copies would push the next pass's exp
            # chain late and stall QK); for the final qb split DVE / ACT so
            # two norms run in parallel and the tail is shortest. Output DMA
            # per s-block, alternating Sync / GpSimd issue queues.
            final = qb == NQB - 1
            for s in range(NS):
                recip = outp.tile([128, 1], f32, tag=f"recip{s}", name="recip")
                nc.vector.reciprocal(recip, o_ps[s][:, DV:DV + 1])
                o_sb = outp.tile([128, DV], f32, tag=f"osb{s}", name="o_sb")
                if final and s in (1, 2):
                    nc.scalar.activation(
                        out=o_sb, in_=o_ps[s][:, 0:DV],
                        func=mybir.ActivationFunctionType.Copy, scale=recip,
                    )
                else:
                    nc.vector.tensor_scalar_mul(o_sb, o_ps[s][:, 0:DV], recip)
                r0 = qb * QB + s * 128
                eng = nc.sync if s % 2 == 0 else nc.gpsimd
                eng.dma_start(out=o.ap()[r0:r0 + 128, :], in_=o_sb)

    nc.compile()
    return nc


def get_nc():
    global _NC
    if _NC is None:
        _NC = _build()
    return _NC


def make_in_maps(query, key, value):
    query = np.asarray(query, dtype=np.float32)
    key = np.asarray(key, dtype=np.float32)
    value = np.asarray(value, dtype=np.float32)
    bf16 = ml_dtypes.bfloat16

    kT = np.ascontiguousarray(key.T).astype(bf16)       # [256, 8192]
    qT_all = np.ascontiguousarray(query.T)              # [256, 8192] f32
    # col 256 = ones (softmax denominator accumulator); col 257 = zero pad
    # (keeps the PSUM dst 8B-aligned at 258 f32 cols).
    pad = np.zeros((M, 2), dtype=np.float32)
    pad[:, 0] = 1.0
    vext = np.concatenate([value, pad], axis=1)         # [8192, 258]
    # partition-major permute: vP[p, b, j] = vext[b*128 + p, j]
    vP = np.ascontiguousarray(
        vext.reshape(NKB, 128, VW).transpose(1, 0, 2)
    ).astype(bf16)                                      # [128, 64, 258]

    return [
        {
            "qT": np.ascontiguousarray(
                qT_all[:, c * QSH:(c + 1) * QSH]
            ).astype(bf16),
            "kT": kT,
            "vP": vP,
        }
        for c in range(NCORES)
    ]


def run(query, key, value, trace=False):
    nc = get_nc()
    in_maps = make_in_maps(query, key, value)
    res = bass_utils.run_bass_kernel_spmd(
        nc, in_maps, core_ids=list(range(NCORES)), trace=trace,
    )
    out = np.concatenate([res.results[c]["o"] for c in range(NCORES)], axis=0)
    return out, res


def kernel(query, key, value):
    out, _ = run(query, key, value)
    return out


# revision 11
# speedup vs baseline: 1.1238x; 1.1238x over previous
"""Trainium2 Bass kernel for single-head attention.

Problem: query [8192, 256], key [8192, 256], value [8192, 256] (fp32)
  out = softmax(Q @ K.T / sqrt(256)) @ V        -> [8192, 256]

Sharding: query rows split across 8 NeuronCores (1024 rows each);
K / V replicated. Each core computes its row-block independently.

Per-core algorithm (core c):
  - Layout trick: compute S^T [k, q] instead of S [q, k] so that the
    PV matmul needs no transpose:  S^T tile = (K chunk) @ (Q chunk)^T via
    PE matmul with d (head dim) on the contraction/partition axis:
        lhsT = K^T[d_chunk, k_block] (128x128), rhs = Q^T[d_chunk, q_block]
  - Scores ~ N(0,1) after the 1/16 scale, so exp() without max-subtraction
    is numerically safe (max score over 8192 samples ~ 4; exp(4) = 55).
  - P^T = exp(S^T / 16) on the ACT engine (scale fused), bf16 output.
    Two k-blocks' score tiles live in one PSUM pair tile [128, 2, 512]
    (adjacent banks) so ONE ACTIVATE covers both: halves ACT instruction
    count and gives the st-tile WAR chain ~1.2us of slack per super-block
    (separate 688ns exps had ~100ns slack -> periodic 432ns PE stalls).
  - O accumulation: out[q, v] = sum_k P^T[k, q]^T @ Vext[k, v] where Vext
    has a ones column appended -> column 256 accumulates the softmax
    denominator sum_k p. One PSUM accumulation group over all 64 k-blocks.
  - Normalize: O[:, 0:256] * (1 / O[:, 256]) per partition row, DMA out.

All matmul operands are bf16: same PE stream rate as float32r (1 col/
cycle) but LDWEIGHTS gets fast-weight-load (fp32r's fp32_mode=HIGH
disables FWL: ~12-18ns/matmul extra) and input DMA bytes halve.
Measured rms relative error ~3.3e-3 (tolerance 2e-2).

A dummy-matmul warmup chain runs during the initial DMA wait (~7.7-12us)
so the PE HAM clock-gate (cold 1.2 GHz -> warm 2.4 GHz after ~1-2 busy
windows of 3.4us) un-throttles before real work, and the PE is never
idle ahead of it.
"""
import numpy as np
import ml_dtypes
from contextlib import ExitStack

import concourse.bacc as bacc
import concourse.mybir as mybir
import concourse.tile as tile
from concourse import bass_utils

N, M, D, DV = 8192, 8192, 256, 256
NCORES = 8
QSH = N // NCORES        # 1024 query rows per core
QB = 512                 # q block (matmul moving free dim; one PSUM bank)
NQB = QSH // QB          # 2
KB = 128                 # k block (PE partition dim)
NKB = M // KB            # 64
SCALE = 1.0 / 16.0       # 1/sqrt(D)
DCH = D // 128           # 2 chunks of the contraction (head) dim
VW = DV + 2              # V columns + ones (denominator) + pad

_NC = None


def _build():
    f32 = mybir.dt.float32
    bf16 = mybir.dt.bfloat16

    nc = bacc.Bacc("TRN2", target_bir_lowering=False, debug=False)
    qT = nc.dram_tensor("qT", [D, QSH], bf16, kind="ExternalInput")
    kT = nc.dram_tensor("kT", [D, M], bf16, kind="ExternalInput")
    # V pre-permuted host-side to partition-major [128, NKB, 258] so DMA
    # lines are (b-range x 258) contiguous.
    vP = nc.dram_tensor("vP", [128, NKB, VW], bf16, kind="ExternalInput")
    o = nc.dram_tensor("o", [QSH, DV], f32, kind="ExternalOutput")
    # Scratch sink for tiny ring-warming DMAs: the output DMA queues sit
    # idle for ~60us before the final output writes, and a cold DGE ring
    # adds ~1.5us of relatch latency to the first post-idle transfer.
    scr = nc.dram_tensor("scr", [128, 4], bf16)

    kT_r = kT.ap().rearrange("(c p) k -> p c k", p=128)    # [128, 2, 8192]
    qT_r = qT.ap().rearrange("(c p) q -> p c q", p=128)    # [128, 2, 1024]
    v_r = vP.ap()                                          # [128, 64, 258]

    with tile.TileContext(nc) as tc, ExitStack() as ctx:
        sb = ctx.enter_context(tc.tile_pool(name="sb", bufs=1))
        pp = ctx.enter_context(tc.tile_pool(name="pp", bufs=4))
        outp = ctx.enter_context(tc.tile_pool(name="outp", bufs=2))
        # st pair tiles: [128, 2, 512] f32 = 2 adjacent PSUM banks; slice
        # [:, j, :] is exactly one bank (a legal matmul dst). 2 bufs = 4 banks.
        ps_st = ctx.enter_context(tc.tile_pool(name="ps_st", bufs=2, space="PSUM"))
        ps_o = ctx.enter_context(tc.tile_pool(name="ps_o", bufs=1, space="PSUM"))

        kt_sb = sb.tile([128, DCH, M], bf16, tag="kt")
        qt_sb = sb.tile([128, DCH, QSH], bf16, tag="qt")
        v_sb = sb.tile([128, NKB, VW], bf16, tag="v")
        warm = sb.tile([128, 256], bf16, tag="warm")

        # PE warmup: accumulation chain with no DMA dependencies, emitted
        # first so it runs during the initial DMA wait (~7.7us preamble end
        # -> ~12us first data) and starts the HAM activity window early.
        # Its PSUM comes from the ps_st rotation; SB0's second pair tile
        # carries a WAR dep on it -- satisfied long before that QK runs.
        nc.vector.memset(warm, 0.0)
        warm_ps = ps_st.tile([128, 2, QB], f32, tag="st", name="warm_ps")
        NWARM = 18
        for i in range(NWARM):
            nc.tensor.matmul(
                warm_ps[:, 0, 0:256], lhsT=warm[:, 0:128], rhs=warm,
                start=(i == 0), stop=(i == NWARM - 1),
            )

        # Input streaming, in consumption order, issued from two idle engine
        # queues in parallel (each DMA_DIRECT2D occupies its queue
        # ~0.7-1.5us): Sync feeds kt, GpSimd feeds qt + v. DGE packets only
        # start flowing ~2.5us after the first issue, then ~300 GB/s.
        nc.sync.dma_start(out=kt_sb[:, 0, 0:512], in_=kT_r[:, 0, 0:512])
        nc.gpsimd.dma_start(out=qt_sb[:, 0, 0:QB], in_=qT_r[:, 0, 0:QB])
        nc.sync.dma_start(out=kt_sb[:, 1, 0:512], in_=kT_r[:, 1, 0:512])
        nc.gpsimd.dma_start(out=qt_sb[:, 1, 0:QB], in_=qT_r[:, 1, 0:QB])
        nc.sync.dma_start(out=kt_sb[:, :, 512:1024], in_=kT_r[:, :, 512:1024])
        nc.gpsimd.dma_start(out=v_sb[:, 0:4, :], in_=v_r[:, 0:4, :])
        nc.sync.dma_start(out=kt_sb[:, :, 1024:2048], in_=kT_r[:, :, 1024:2048])
        nc.gpsimd.dma_start(out=v_sb[:, 4:12, :], in_=v_r[:, 4:12, :])
        nc.sync.dma_start(out=kt_sb[:, :, 2048:4096], in_=kT_r[:, :, 2048:4096])
        nc.gpsimd.dma_start(out=v_sb[:, 12:28, :], in_=v_r[:, 12:28, :])
        nc.sync.dma_start(out=kt_sb[:, :, 4096:8192], in_=kT_r[:, :, 4096:8192])
        nc.gpsimd.dma_start(out=qt_sb[:, :, QB:QSH], in_=qT_r[:, :, QB:QSH])
        nc.gpsimd.dma_start(out=v_sb[:, 28:44, :], in_=v_r[:, 28:44, :])
        nc.gpsimd.dma_start(out=v_sb[:, 44:64, :], in_=v_r[:, 44:64, :])

        SB = 4  # kb super-block: longer same-type PE runs, fewer transitions
        NS = QB // 128
        NPAIR = SB // 2
        for qb in range(NQB):
            qsl = slice(qb * QB, (qb + 1) * QB)
            o_ps = [ps_o.tile([128, VW], f32, tag=f"o{s}", name=f"o_ps{s}") for s in range(NS)]

            def emit_pv_half(pts, kb0, pr, last):
                # s-major in the last half of the last super-block: each o_ps
                # accumulation closes as early as possible, so normalization +
                # output DMA overlap the remaining PV matmuls.
                js = (pr * 2, pr * 2 + 1)
                if last:
                    order = [(j, s) for s in range(NS) for j in js]
                else:
                    order = [(j, s) for j in js for s in range(NS)]
                for j, s in order:
                    kb = kb0 + j
                    nc.tensor.matmul(
                        o_ps[s],
                        lhsT=pts[j // 2][:, j % 2, s * 128:(s + 1) * 128],
                        rhs=v_sb[:, kb, :],
                        start=(kb == 0),
                        stop=(kb == NKB - 1),
                    )

            def emit_qk_pair(st, kb0, pr):
                # Interleave the pair's two k-blocks: consecutive PE matmuls
                # hit different PSUM banks (avoids same-bank accumulate
                # turnaround between a group's start and stop matmul).
                for c in range(DCH):
                    for j in (0, 1):
                        kb = kb0 + pr * 2 + j
                        ksl = slice(kb * KB, (kb + 1) * KB)
                        nc.tensor.matmul(
                            st[:, j, :],
                            lhsT=kt_sb[:, c, ksl],
                            rhs=qt_sb[:, c, qsl],
                            start=(c == 0),
                            stop=(c == DCH - 1),
                        )

            # Software-pipelined emission, interleaved at PAIR granularity:
            #   QKa(i) exp_a(i) PVa(i-1) QKb(i) exp_b(i) PVb(i-1)
            # Two purposes: (1) by the time PE reaches a PV group its exp
            # finished ~a super-block ago; (2) each wait-carrying QK start
            # matmul sits ~900ns after its exp semaphore arrives -- the PE NX
            # decodes ~860ns ahead, and a wait unsatisfied at decode time
            # breaks the matmul pipeline for ~2 issue slots (measured 432ns).
            prev_pv = None
            for kb0 in range(0, NKB, SB):
                sts = [
                    ps_st.tile([128, 2, QB], f32, tag="st", name="st")
                    for _ in range(NPAIR)
                ]
                for pr in range(NPAIR):
                    emit_qk_pair(sts[pr], kb0, pr)
                    p_t = pp.tile([128, 2, QB], bf16, tag="p", name="p_t")
                    nc.scalar.activation(
                        out=p_t, in_=sts[pr],
                        func=mybir.ActivationFunctionType.Exp, scale=SCALE,
                    )
                    if pr == 0 and qb == NQB - 1 and kb0 == NKB - SB:
                        # warm the output DMA rings ~1.5 super-blocks early
                        nc.sync.dma_start(out=scr.ap()[:, 0:2], in_=p_t[:, 0, 0:2])
                        nc.gpsimd.dma_start(out=scr.ap()[:, 2:4], in_=p_t[:, 0, 2:4])
                    if prev_pv is not None:
                        emit_pv_half(prev_pv[0], prev_pv[1], pr, last=False)
                    if pr == 0:
                        pts = []
                    pts.append(p_t)
                prev_pv = (pts, kb0)
            for pr in range(NPAIR):
                emit_pv_half(prev_pv[0], prev_pv[1], pr, last=True)
            # Normalize: recip on DVE as each o_ps closes. For qb0 keep ALL
            # multiplies on DVE (ACT copies would push the next pass's exp
            # chain late and stall QK); for the final qb split DVE / ACT so
            # two norms run in parallel and the tail is shortest. Output DMA
            # per s-block, alternating Sync / GpSimd issue queues.
            final = qb == NQB - 1
            for s in range(NS):
                recip = outp.tile([128, 1], f32, tag=f"recip{s}", name="recip")
                nc.vector.reciprocal(recip, o_ps[s][:, DV:DV + 1])
                o_sb = outp.tile([128, DV], f32, tag=f"osb{s}", name="o_sb")
                if final and s in (1, 2):
                    nc.scalar.activation(
                        out=o_sb, in_=o_ps[s][:, 0:DV],
                        func=mybir.ActivationFunctionType.Copy, scale=recip,
                    )
                else:
                    nc.vector.tensor_scalar_mul(o_sb, o_ps[s][:, 0:DV], recip)
                r0 = qb * QB + s * 128
                eng = nc.sync if s % 2 == 0 else nc.gpsimd
                eng.dma_start(out=o.ap()[r0:r0 + 128, :], in_=o_sb)

    nc.compile()
    return nc


def get_nc():
    global _NC
    if _NC is None:
        _NC = _build()
    return _NC


def make_in_maps(query, key, value):
    query = np.asarray(query, dtype=np.float32)
    key = np.asarray(key, dtype=np.float32)
    value = np.asarray(value, dtype=np.float32)
    bf16 = ml_dtypes.bfloat16

    kT = np.ascontiguousarray(key.T).astype(bf16)       # [256, 8192]
    qT_all = np.ascontiguousarray(query.T)              # [256, 8192] f32
    # col 256 = ones (softmax denominator accumulator); col 257 = zero pad
    # (keeps the PSUM dst 8B-aligned at 258 f32 cols).
    pad = np.zeros((M, 2), dtype=np.float32)
    pad[:, 0] = 1.0
    vext = np.concatenate([value, pad], axis=1)         # [8192, 258]
    # partition-major permute: vP[p, b, j] = vext[b*128 + p, j]
    vP = np.ascontiguousarray(
        vext.reshape(NKB, 128, VW).transpose(1, 0, 2)
    ).astype(bf16)                                      # [128, 64, 258]

    return [
        {
            "qT": np.ascontiguousarray(
                qT_all[:, c * QSH:(c + 1) * QSH]
            ).astype(bf16),
            "kT": kT,
            "vP": vP,
        }
        for c in range(NCORES)
    ]


def run(query, key, value, trace=False):
    nc = get_nc()
    in_maps = make_in_maps(query, key, value)
    res = bass_utils.run_bass_kernel_spmd(
        nc, in_maps, core_ids=list(range(NCORES)), trace=trace,
    )
    out = np.concatenate([res.results[c]["o"] for c in range(NCORES)], axis=0)
    return out, res


def kernel(query, key, value):
    out, _ = run(query, key, value)
    return out


# revision 14
# speedup vs baseline: 1.1929x; 1.0615x over previous
"""Trainium2 Bass kernel for single-head attention.

Problem: query [8192, 256], key [8192, 256], value [8192, 256] (fp32)
  out = softmax(Q @ K.T / sqrt(256)) @ V        -> [8192, 256]

Sharding: query rows split across 8 NeuronCores (1024 rows each);
K / V replicated. Each core computes its row-block independently.

Per-core algorithm (core c):
  - Layout trick: compute S^T [k, q] instead of S [q, k] so that the
    PV matmul needs no transpose:  S^T tile = (K chunk) @ (Q chunk)^T via
    PE matmul with d (head dim) on the contraction/partition axis:
        lhsT = K^T[d_chunk, k_block] (128x128), rhs = Q^T[d_chunk, q_block]
  - Scores ~ N(0,1) after the 1/16 scale, so exp() without max-subtraction
    is numerically safe (max score over 8192 samples ~ 4; exp(4) = 55).
  - P^T = exp(S^T / 16) on the ACT engine (scale fused), bf16 output.
    Two k-blocks' score tiles live in one PSUM pair tile [128, 2, 512]
    (adjacent banks) so ONE ACTIVATE covers both: halves ACT instruction
    count and gives the st-tile WAR chain ~1.2us of slack per super-block
    (separate 688ns exps had ~100ns slack -> periodic 432ns PE stalls).
  - O accumulation: out[q, v] = sum_k P^T[k, q]^T @ Vext[k, v] where Vext
    has a ones column appended -> column 256 accumulates the softmax
    denominator sum_k p. One PSUM accumulation group over all 64 k-blocks.
  - Normalize: O[:, 0:256] * (1 / O[:, 256]) per partition row, DMA out.

All matmul operands are bf16: same PE stream rate as float32r (1 col/
cycle) but LDWEIGHTS gets fast-weight-load (fp32r's fp32_mode=HIGH
disables FWL: ~12-18ns/matmul extra) and input DMA bytes halve.
Measured rms relative error ~3.3e-3 (tolerance 2e-2).

A dummy-matmul warmup chain runs during the initial DMA wait (~7.7-12us)
so the PE HAM clock-gate (cold 1.2 GHz -> warm 2.4 GHz after ~1-2 busy
windows of 3.4us) un-throttles before real work, and the PE is never
idle ahead of it.
"""
import numpy as np
import ml_dtypes
from contextlib import ExitStack

import concourse.bacc as bacc
import concourse.mybir as mybir
import concourse.tile as tile
from concourse import bass_utils

N, M, D, DV = 8192, 8192, 256, 256
NCORES = 8
QSH = N // NCORES        # 1024 query rows per core
QB = 512                 # q block (matmul moving free dim; one PSUM bank)
NQB = QSH // QB          # 2
KB = 128                 # k block (PE partition dim)
NKB = M // KB            # 64
SCALE = 1.0 / 16.0       # 1/sqrt(D)
DCH = D // 128           # 2 chunks of the contraction (head) dim
VW = DV + 2              # V columns + ones (denominator) + pad

_NC = None


def _build():
    f32 = mybir.dt.float32
    bf16 = mybir.dt.bfloat16

    nc = bacc.Bacc("TRN2", target_bir_lowering=False, debug=False)
    qT = nc.dram_tensor("qT", [D, QSH], bf16, kind="ExternalInput")
    kT = nc.dram_tensor("kT", [D, M], bf16, kind="ExternalInput")
    # V pre-permuted host-side to partition-major [128, NKB, 258] so DMA
    # lines are (b-range x 258) contiguous.
    vP = nc.dram_tensor("vP", [128, NKB, VW], bf16, kind="ExternalInput")
    o = nc.dram_tensor("o", [QSH, DV], f32, kind="ExternalOutput")
    # Scratch sink for tiny ring-warming DMAs: the output DMA queues sit
    # idle for ~60us before the final output writes, and a cold DGE ring
    # adds ~1.5us of relatch latency to the first post-idle transfer.
    scr = nc.dram_tensor("scr", [128, 4], bf16)

    kT_r = kT.ap().rearrange("(c p) k -> p c k", p=128)    # [128, 2, 8192]
    qT_r = qT.ap().rearrange("(c p) q -> p c q", p=128)    # [128, 2, 1024]
    v_r = vP.ap()                                          # [128, 64, 258]

    with tile.TileContext(nc) as tc, ExitStack() as ctx:
        sb = ctx.enter_context(tc.tile_pool(name="sb", bufs=1))
        pp = ctx.enter_context(tc.tile_pool(name="pp", bufs=4))
        outp = ctx.enter_context(tc.tile_pool(name="outp", bufs=2))
        # st pair tiles: [128, 2, 512] f32 = 2 adjacent PSUM banks; slice
        # [:, j, :] is exactly one bank (a legal matmul dst). 2 bufs = 4 banks.
        ps_st = ctx.enter_context(tc.tile_pool(name="ps_st", bufs=2, space="PSUM"))
        ps_o = ctx.enter_context(tc.tile_pool(name="ps_o", bufs=1, space="PSUM"))

        kt_sb = sb.tile([128, DCH, M], bf16, tag="kt")
        qt_sb = sb.tile([128, DCH, QSH], bf16, tag="qt")
        v_sb = sb.tile([128, NKB, VW], bf16, tag="v")
        warm = sb.tile([128, 256], bf16, tag="warm")

        # PE warmup: accumulation chain with no DMA dependencies, emitted
        # first so it runs during the initial DMA wait (~7.7us preamble end
        # -> ~12us first data) and starts the HAM activity window early.
        # Its PSUM comes from the ps_st rotation; SB0's second pair tile
        # carries a WAR dep on it -- satisfied long before that QK runs.
        nc.vector.memset(warm, 0.0)
        warm_ps = ps_st.tile([128, 2, QB], f32, tag="st", name="warm_ps")
        NWARM = 18
        for i in range(NWARM):
            nc.tensor.matmul(
                warm_ps[:, 0, 0:256], lhsT=warm[:, 0:128], rhs=warm,
                start=(i == 0), stop=(i == NWARM - 1),
            )

        # Input streaming, in consumption order, issued from two idle engine
        # queues in parallel (each DMA_DIRECT2D occupies its queue
        # ~0.7-1.5us): Sync feeds kt, GpSimd feeds qt + v. DGE packets only
        # start flowing ~2.5us after the first issue, then ~300 GB/s.
        nc.sync.dma_start(out=kt_sb[:, :, 0:512], in_=kT_r[:, :, 0:512])
        nc.gpsimd.dma_start(out=qt_sb[:, :, 0:QB], in_=qT_r[:, :, 0:QB])
        nc.sync.dma_start(out=kt_sb[:, :, 512:1024], in_=kT_r[:, :, 512:1024])
        nc.gpsimd.dma_start(out=v_sb[:, 0:4, :], in_=v_r[:, 0:4, :])
        nc.sync.dma_start(out=kt_sb[:, :, 1024:2048], in_=kT_r[:, :, 1024:2048])
        nc.gpsimd.dma_start(out=v_sb[:, 4:12, :], in_=v_r[:, 4:12, :])
        nc.sync.dma_start(out=kt_sb[:, :, 2048:4096], in_=kT_r[:, :, 2048:4096])
        nc.gpsimd.dma_start(out=v_sb[:, 12:28, :], in_=v_r[:, 12:28, :])
        nc.sync.dma_start(out=kt_sb[:, :, 4096:8192], in_=kT_r[:, :, 4096:8192])
        nc.gpsimd.dma_start(out=qt_sb[:, :, QB:QSH], in_=qT_r[:, :, QB:QSH])
        nc.gpsimd.dma_start(out=v_sb[:, 28:44, :], in_=v_r[:, 28:44, :])
        nc.gpsimd.dma_start(out=v_sb[:, 44:64, :], in_=v_r[:, 44:64, :])

        SB = 4  # kb super-block: longer same-type PE runs, fewer transitions
        NS = QB // 128
        NPAIR = SB // 2
        for qb in range(NQB):
            qsl = slice(qb * QB, (qb + 1) * QB)
            o_ps = [ps_o.tile([128, VW], f32, tag=f"o{s}", name=f"o_ps{s}") for s in range(NS)]

            def emit_pv_half(pts, kb0, pr, last):
                # s-major in the last half of the last super-block: each o_ps
                # accumulation closes as early as possible, so normalization +
                # output DMA overlap the remaining PV matmuls.
                js = (pr * 2, pr * 2 + 1)
                if last:
                    order = [(j, s) for s in range(NS) for j in js]
                else:
                    order = [(j, s) for j in js for s in range(NS)]
                for j, s in order:
                    kb = kb0 + j
                    nc.tensor.matmul(
                        o_ps[s],
                        lhsT=pts[j // 2][:, j % 2, s * 128:(s + 1) * 128],
                        rhs=v_sb[:, kb, :],
                        start=(kb == 0),
                        stop=(kb == NKB - 1),
                    )

            def emit_qk_pair(st, kb0, pr):
                # Interleave the pair's two k-blocks: consecutive PE matmuls
                # hit different PSUM banks (avoids same-bank accumulate
                # turnaround between a group's start and stop matmul).
                for c in range(DCH):
                    for j in (0, 1):
                        kb = kb0 + pr * 2 + j
                        ksl = slice(kb * KB, (kb + 1) * KB)
                        nc.tensor.matmul(
                            st[:, j, :],
                            lhsT=kt_sb[:, c, ksl],
                            rhs=qt_sb[:, c, qsl],
                            start=(c == 0),
                            stop=(c == DCH - 1),
                        )

            # Software-pipelined emission, interleaved at PAIR granularity:
            #   QKa(i) exp_a(i) PVa(i-1) QKb(i) exp_b(i) PVb(i-1)
            # Two purposes: (1) by the time PE reaches a PV group its exp
            # finished ~a super-block ago; (2) each wait-carrying QK start
            # matmul sits ~900ns after its exp semaphore arrives -- the PE NX
            # decodes ~860ns ahead, and a wait unsatisfied at decode time
            # breaks the matmul pipeline for ~2 issue slots (measured 432ns).
            prev_pv = None
            for kb0 in range(0, NKB, SB):
                sts = [
                    ps_st.tile([128, 2, QB], f32, tag="st", name="st")
                    for _ in range(NPAIR)
                ]
                for pr in range(NPAIR):
                    emit_qk_pair(sts[pr], kb0, pr)
                    p_t = pp.tile([128, 2, QB], bf16, tag="p", name="p_t")
                    nc.scalar.activation(
                        out=p_t, in_=sts[pr],
                        func=mybir.ActivationFunctionType.Exp, scale=SCALE,
                    )
                    if pr == 0 and qb == NQB - 1 and kb0 == NKB - SB:
                        # warm the output DMA ring ~1.5 super-blocks early
                        nc.sync.dma_start(out=scr.ap()[:, 0:2], in_=p_t[:, 0, 0:2])
                    if prev_pv is not None:
                        emit_pv_half(prev_pv[0], prev_pv[1], pr, last=False)
                    if pr == 0:
                        pts = []
                    pts.append(p_t)
                prev_pv = (pts, kb0)
            for pr in range(NPAIR):
                emit_pv_half(prev_pv[0], prev_pv[1], pr, last=True)
            # Normalize: recips on DVE as each o_ps closes; multiplies split
            # DVE / ACT so two run in parallel. At the qb0 boundary ACT gets
            # only ONE copy (more would push qb1's exp chain late and stall
            # QK). Output pairs (two s-blocks in one adjacent tile) need just
            # 2 Sync-issued DMAs per pass -- NEVER GpSimd: its software-DGE
            # path takes ~4us per post-idle transfer and stalls teardown.
            final = qb == NQB - 1
            act_set = (1, 2) if final else (1,)
            recips, halves = [], []
            o_pair = None
            for s in range(NS):
                recip = outp.tile([128, 1], f32, tag=f"recip{s}", name="recip")
                nc.vector.reciprocal(recip, o_ps[s][:, DV:DV + 1])
                if s % 2 == 0:
                    o_pair = outp.tile([128, 2, DV], f32, tag=f"osb{s // 2}", name="o_pair")
                dst = o_pair[:, s % 2, :]
                if s in act_set:
                    nc.scalar.activation(
                        out=dst, in_=o_ps[s][:, 0:DV],
                        func=mybir.ActivationFunctionType.Copy, scale=recip,
                    )
                else:
                    nc.vector.tensor_scalar_mul(dst, o_ps[s][:, 0:DV], recip)
                if s % 2 == 1:
                    r0 = qb * QB + (s - 1) * 128
                    d = o.ap()[r0:r0 + 256, :].rearrange("(s p) v -> p s v", p=128)
                    nc.sync.dma_start(out=d, in_=o_pair)

    nc.compile()
    return nc


def get_nc():
    global _NC
    if _NC is None:
        _NC = _build()
    return _NC


def make_in_maps(query, key, value):
    query = np.asarray(query, dtype=np.float32)
    key = np.asarray(key, dtype=np.float32)
    value = np.asarray(value, dtype=np.float32)
    bf16 = ml_dtypes.bfloat16

    kT = np.ascontiguousarray(key.T).astype(bf16)       # [256, 8192]
    qT_all = np.ascontiguousarray(query.T)              # [256, 8192] f32
    # col 256 = ones (softmax denominator accumulator); col 257 = zero pad
    # (keeps the PSUM dst 8B-aligned at 258 f32 cols).
    pad = np.zeros((M, 2), dtype=np.float32)
    pad[:, 0] = 1.0
    vext = np.concatenate([value, pad], axis=1)         # [8192, 258]
    # partition-major permute: vP[p, b, j] = vext[b*128 + p, j]
    vP = np.ascontiguousarray(
        vext.reshape(NKB, 128, VW).transpose(1, 0, 2)
    ).astype(bf16)                                      # [128, 64, 258]

    return [
        {
            "qT": np.ascontiguousarray(
                qT_all[:, c * QSH:(c + 1) * QSH]
            ).astype(bf16),
            "kT": kT,
            "vP": vP,
        }
        for c in range(NCORES)
    ]


def run(query, key, value, trace=False):
    nc = get_nc()
    in_maps = make_in_maps(query, key, value)
    res = bass_utils.run_bass_kernel_spmd(
        nc, in_maps, core_ids=list(range(NCORES)), trace=trace,
    )
    out = np.concatenate([res.results[c]["o"] for c in range(NCORES)], axis=0)
    return out, res


def kernel(query, key, value):
    out, _ = run(query, key, value)
    return out


# revision 15
# speedup vs baseline: 1.1980x; 1.0043x over previous
"""Trainium2 Bass kernel for single-head attention.

Problem: query [8192, 256], key [8192, 256], value [8192, 256] (fp32)
  out = softmax(Q @ K.T / sqrt(256)) @ V        -> [8192, 256]

Sharding: query rows split across 8 NeuronCores (1024 rows each);
K / V replicated. Each core computes its row-block independently.

Per-core algorithm (core c):
  - Layout trick: compute S^T [k, q] instead of S [q, k] so that the
    PV matmul needs no transpose:  S^T tile = (K chunk) @ (Q chunk)^T via
    PE matmul with d (head dim) on the contraction/partition axis:
        lhsT = K^T[d_chunk, k_block] (128x128), rhs = Q^T[d_chunk, q_block]
  - Scores ~ N(0,1) after the 1/16 scale, so exp() without max-subtraction
    is numerically safe (max score over 8192 samples ~ 4; exp(4) = 55).
  - P^T = exp(S^T / 16) on the ACT engine (scale fused), bf16 output.
    Two k-blocks' score tiles live in one PSUM pair tile [128, 2, 512]
    (adjacent banks) so ONE ACTIVATE covers both: halves ACT instruction
    count and gives the st-tile WAR chain ~1.2us of slack per super-block
    (separate 688ns exps had ~100ns slack -> periodic 432ns PE stalls).
  - O accumulation: out[q, v] = sum_k P^T[k, q]^T @ Vext[k, v] where Vext
    has a ones column appended -> column 256 accumulates the softmax
    denominator sum_k p. One PSUM accumulation group over all 64 k-blocks.
  - Normalize: O[:, 0:256] * (1 / O[:, 256]) per partition row, DMA out.

All matmul operands are bf16: same PE stream rate as float32r (1 col/
cycle) but LDWEIGHTS gets fast-weight-load (fp32r's fp32_mode=HIGH
disables FWL: ~12-18ns/matmul extra) and input DMA bytes halve.
Measured rms relative error ~3.3e-3 (tolerance 2e-2).

A dummy-matmul warmup chain runs during the initial DMA wait (~7.7-12us)
so the PE HAM clock-gate (cold 1.2 GHz -> warm 2.4 GHz after ~1-2 busy
windows of 3.4us) un-throttles before real work, and the PE is never
idle ahead of it.
"""
import numpy as np
import ml_dtypes
from contextlib import ExitStack

import concourse.bacc as bacc
import concourse.mybir as mybir
import concourse.tile as tile
from concourse import bass_utils

N, M, D, DV = 8192, 8192, 256, 256
NCORES = 8
QSH = N // NCORES        # 1024 query rows per core
QB = 512                 # q block (matmul moving free dim; one PSUM bank)
NQB = QSH // QB          # 2
KB = 128                 # k block (PE partition dim)
NKB = M // KB            # 64
SCALE = 1.0 / 16.0       # 1/sqrt(D)
DCH = D // 128           # 2 chunks of the contraction (head) dim
VW = DV + 2              # V columns + ones (denominator) + pad

_NC = None


def _build():
    f32 = mybir.dt.float32
    bf16 = mybir.dt.bfloat16

    nc = bacc.Bacc("TRN2", target_bir_lowering=False, debug=False)
    qT = nc.dram_tensor("qT", [D, QSH], bf16, kind="ExternalInput")
    kT = nc.dram_tensor("kT", [D, M], bf16, kind="ExternalInput")
    # V pre-permuted host-side to partition-major [128, NKB, 258] so DMA
    # lines are (b-range x 258) contiguous.
    vP = nc.dram_tensor("vP", [128, NKB, VW], bf16, kind="ExternalInput")
    o = nc.dram_tensor("o", [QSH, DV], f32, kind="ExternalOutput")
    # Scratch sink for tiny ring-warming DMAs: the output DMA queues sit
    # idle for ~60us before the final output writes, and a cold DGE ring
    # adds ~1.5us of relatch latency to the first post-idle transfer.
    scr = nc.dram_tensor("scr", [128, 4], bf16)

    kT_r = kT.ap().rearrange("(c p) k -> p c k", p=128)    # [128, 2, 8192]
    qT_r = qT.ap().rearrange("(c p) q -> p c q", p=128)    # [128, 2, 1024]
    v_r = vP.ap()                                          # [128, 64, 258]

    with tile.TileContext(nc) as tc, ExitStack() as ctx:
        sb = ctx.enter_context(tc.tile_pool(name="sb", bufs=1))
        pp = ctx.enter_context(tc.tile_pool(name="pp", bufs=4))
        outp = ctx.enter_context(tc.tile_pool(name="outp", bufs=2))
        # st pair tiles: [128, 2, 512] f32 = 2 adjacent PSUM banks; slice
        # [:, j, :] is exactly one bank (a legal matmul dst). 2 bufs = 4 banks.
        ps_st = ctx.enter_context(tc.tile_pool(name="ps_st", bufs=2, space="PSUM"))
        ps_o = ctx.enter_context(tc.tile_pool(name="ps_o", bufs=1, space="PSUM"))

        kt_sb = sb.tile([128, DCH, M], bf16, tag="kt")
        qt_sb = sb.tile([128, DCH, QSH], bf16, tag="qt")
        v_sb = sb.tile([128, NKB, VW], bf16, tag="v")
        warm = sb.tile([128, 256], bf16, tag="warm")

        # PE warmup: accumulation chain with no DMA dependencies, emitted
        # first so it runs during the initial DMA wait (~7.7us preamble end
        # -> ~12us first data) and starts the HAM activity window early.
        # Its PSUM comes from the ps_st rotation; SB0's second pair tile
        # carries a WAR dep on it -- satisfied long before that QK runs.
        nc.vector.memset(warm, 0.0)
        warm_ps = ps_st.tile([128, 2, QB], f32, tag="st", name="warm_ps")
        NWARM = 18
        for i in range(NWARM):
            nc.tensor.matmul(
                warm_ps[:, 0, 0:256], lhsT=warm[:, 0:128], rhs=warm,
                start=(i == 0), stop=(i == NWARM - 1),
            )

        # Input streaming, in consumption order, issued from two idle engine
        # queues in parallel (each DMA_DIRECT2D occupies its queue
        # ~0.7-1.5us): Sync feeds kt, GpSimd feeds qt + v. DGE packets only
        # start flowing ~2.5us after the first issue, then ~300 GB/s.
        nc.sync.dma_start(out=kt_sb[:, :, 0:512], in_=kT_r[:, :, 0:512])
        nc.gpsimd.dma_start(out=qt_sb[:, :, 0:QB], in_=qT_r[:, :, 0:QB])
        nc.sync.dma_start(out=kt_sb[:, :, 512:1024], in_=kT_r[:, :, 512:1024])
        nc.gpsimd.dma_start(out=v_sb[:, 0:4, :], in_=v_r[:, 0:4, :])
        nc.sync.dma_start(out=kt_sb[:, :, 1024:1536], in_=kT_r[:, :, 1024:1536])
        nc.gpsimd.dma_start(out=v_sb[:, 4:8, :], in_=v_r[:, 4:8, :])
        nc.sync.dma_start(out=kt_sb[:, :, 1536:2048], in_=kT_r[:, :, 1536:2048])
        nc.gpsimd.dma_start(out=v_sb[:, 8:16, :], in_=v_r[:, 8:16, :])
        nc.sync.dma_start(out=kt_sb[:, :, 2048:3072], in_=kT_r[:, :, 2048:3072])
        nc.gpsimd.dma_start(out=qt_sb[:, :, QB:QSH], in_=qT_r[:, :, QB:QSH])
        nc.sync.dma_start(out=kt_sb[:, :, 3072:4096], in_=kT_r[:, :, 3072:4096])
        nc.gpsimd.dma_start(out=v_sb[:, 16:28, :], in_=v_r[:, 16:28, :])
        nc.sync.dma_start(out=kt_sb[:, :, 4096:6144], in_=kT_r[:, :, 4096:6144])
        nc.gpsimd.dma_start(out=v_sb[:, 28:44, :], in_=v_r[:, 28:44, :])
        nc.sync.dma_start(out=kt_sb[:, :, 6144:8192], in_=kT_r[:, :, 6144:8192])
        nc.gpsimd.dma_start(out=v_sb[:, 44:64, :], in_=v_r[:, 44:64, :])

        SB = 4  # kb super-block: longer same-type PE runs, fewer transitions
        NS = QB // 128
        NPAIR = SB // 2
        for qb in range(NQB):
            qsl = slice(qb * QB, (qb + 1) * QB)
            o_ps = [ps_o.tile([128, VW], f32, tag=f"o{s}", name=f"o_ps{s}") for s in range(NS)]

            def emit_pv_half(pts, kb0, pr, last):
                # s-major in the last half of the last super-block: each o_ps
                # accumulation closes as early as possible, so normalization +
                # output DMA overlap the remaining PV matmuls.
                js = (pr * 2, pr * 2 + 1)
                if last:
                    order = [(j, s) for s in range(NS) for j in js]
                else:
                    order = [(j, s) for j in js for s in range(NS)]
                for j, s in order:
                    kb = kb0 + j
                    nc.tensor.matmul(
                        o_ps[s],
                        lhsT=pts[j // 2][:, j % 2, s * 128:(s + 1) * 128],
                        rhs=v_sb[:, kb, :],
                        start=(kb == 0),
                        stop=(kb == NKB - 1),
                    )

            def emit_qk_pair(st, kb0, pr):
                # Interleave the pair's two k-blocks: consecutive PE matmuls
                # hit different PSUM banks (avoids same-bank accumulate
                # turnaround between a group's start and stop matmul).
                for c in range(DCH):
                    for j in (0, 1):
                        kb = kb0 + pr * 2 + j
                        ksl = slice(kb * KB, (kb + 1) * KB)
                        nc.tensor.matmul(
                            st[:, j, :],
                            lhsT=kt_sb[:, c, ksl],
                            rhs=qt_sb[:, c, qsl],
                            start=(c == 0),
                            stop=(c == DCH - 1),
                        )

            # Software-pipelined emission, interleaved at PAIR granularity:
            #   QKa(i) exp_a(i) PVa(i-1) QKb(i) exp_b(i) PVb(i-1)
            # Two purposes: (1) by the time PE reaches a PV group its exp
            # finished ~a super-block ago; (2) each wait-carrying QK start
            # matmul sits ~900ns after its exp semaphore arrives -- the PE NX
            # decodes ~860ns ahead, and a wait unsatisfied at decode time
            # breaks the matmul pipeline for ~2 issue slots (measured 432ns).
            prev_pv = None
            for kb0 in range(0, NKB, SB):
                sts = [
                    ps_st.tile([128, 2, QB], f32, tag="st", name="st")
                    for _ in range(NPAIR)
                ]
                for pr in range(NPAIR):
                    emit_qk_pair(sts[pr], kb0, pr)
                    p_t = pp.tile([128, 2, QB], bf16, tag="p", name="p_t")
                    nc.scalar.activation(
                        out=p_t, in_=sts[pr],
                        func=mybir.ActivationFunctionType.Exp, scale=SCALE,
                    )
                    if pr == 0 and qb == NQB - 1 and kb0 == NKB - SB:
                        # warm the output DMA ring ~1.5 super-blocks early
                        nc.sync.dma_start(out=scr.ap()[:, 0:2], in_=p_t[:, 0, 0:2])
                    if prev_pv is not None:
                        emit_pv_half(prev_pv[0], prev_pv[1], pr, last=False)
                    if pr == 0:
                        pts = []
                    pts.append(p_t)
                prev_pv = (pts, kb0)
            for pr in range(NPAIR):
                emit_pv_half(prev_pv[0], prev_pv[1], pr, last=True)
            # Normalize: recips on DVE as each o_ps closes; multiplies split
            # DVE / ACT so two run in parallel. At the qb0 boundary ACT gets
            # only ONE copy (more would push qb1's exp chain late and stall
            # QK). Output pairs (two s-blocks in one adjacent tile) need just
            # 2 Sync-issued DMAs per pass -- NEVER GpSimd: its software-DGE
            # path takes ~4us per post-idle transfer and stalls teardown.
            final = qb == NQB - 1
            act_set = (1, 2) if final else (1,)
            recips, halves = [], []
            o_pair = None
            for s in range(NS):
                recip = outp.tile([128, 1], f32, tag=f"recip{s}", name="recip")
                nc.vector.reciprocal(recip, o_ps[s][:, DV:DV + 1])
                if s % 2 == 0:
                    o_pair = outp.tile([128, 2, DV], f32, tag=f"osb{s // 2}", name="o_pair")
                dst = o_pair[:, s % 2, :]
                if s in act_set:
                    nc.scalar.activation(
                        out=dst, in_=o_ps[s][:, 0:DV],
                        func=mybir.ActivationFunctionType.Copy, scale=recip,
                    )
                else:
                    nc.vector.tensor_scalar_mul(dst, o_ps[s][:, 0:DV], recip)
                if s % 2 == 1:
                    r0 = qb * QB + (s - 1) * 128
                    d = o.ap()[r0:r0 + 256, :].rearrange("(s p) v -> p s v", p=128)
                    nc.sync.dma_start(out=d, in_=o_pair)

    nc.compile()
    return nc


def get_nc():
    global _NC
    if _NC is None:
        _NC = _build()
    return _NC


def make_in_maps(query, key, value):
    query = np.asarray(query, dtype=np.float32)
    key = np.asarray(key, dtype=np.float32)
    value = np.asarray(value, dtype=np.float32)
    bf16 = ml_dtypes.bfloat16

    kT = np.ascontiguousarray(key.T).astype(bf16)       # [256, 8192]
    qT_all = np.ascontiguousarray(query.T)              # [256, 8192] f32
    # col 256 = ones (softmax denominator accumulator); col 257 = zero pad
    # (keeps the PSUM dst 8B-aligned at 258 f32 cols).
    pad = np.zeros((M, 2), dtype=np.float32)
    pad[:, 0] = 1.0
    vext = np.concatenate([value, pad], axis=1)         # [8192, 258]
    # partition-major permute: vP[p, b, j] = vext[b*128 + p, j]
    vP = np.ascontiguousarray(
        vext.reshape(NKB, 128, VW).transpose(1, 0, 2)
    ).astype(bf16)                                      # [128, 64, 258]

    return [
        {
            "qT": np.ascontiguousarray(
                qT_all[:, c * QSH:(c + 1) * QSH]
            ).astype(bf16),
            "kT": kT,
            "vP": vP,
        }
        for c in range(NCORES)
    ]


def run(query, key, value, trace=False):
    nc = get_nc()
    in_maps = make_in_maps(query, key, value)
    res = bass_utils.run_bass_kernel_spmd(
        nc, in_maps, core_ids=list(range(NCORES)), trace=trace,
    )
    out = np.concatenate([res.results[c]["o"] for c in range(NCORES)], axis=0)
    return out, res


def kernel(query, key, value):
    out, _ = run(query, key, value)
    return out
